# revision 2
# baseline (speedup 1.0000x reference)
"""Bidirectional Mamba (MixerModel) Trainium2 kernel, v2.

Sharding: data-parallel over batch - 8 batch elements -> 8 NeuronCores; each
core runs the full 2-direction x 4-layer model for its batch element. The
backward direction consumes a host-flipped input; the softmax pool is
order-invariant so nothing is unflipped. Host stacks the per-core [64] rows.

v2 is an instruction-count rewrite of the working v1 (7128 -> ~1.2k instrs):
  * Both directions live in one tile: rows 0:64 / 64:128 for d_model stages,
    free-axis blocks [dir0 | dir1] for d_inner stages, so every elementwise
    instruction uses all 128 lanes and covers both directions at once.
  * Selective scan runs at full T with a boundary-column trick: each state's
    lane is [dir0(2048) | 1 zeroed col | dir1(2048)], so one
    tensor_tensor_scan instruction covers both directions (the zero column
    resets the recurrence exactly) - 16 scans/layer total, no chunk carries.
  * B/C broadcasts to 128 partitions go through a DRAM round trip: bcs rows
    are written once per layer, then one 4-dim DMA per 4-state group reads
    them back replicated across partitions (SBUF src APs cannot have a
    0-stride partition dim; DRAM src APs can).
  * dbx/da for all states of a group are computed by single wide DVE ops
    using stride-0 repeat APs (u repeated over states, A repeated over time).
  * Depthwise conv = 4 scalar_tensor_tensor FMAs per direction.
  * All matmuls run in bf16 (1 cycle/row on the PE).
  * All inputs packed into one f32 + one bf16 blob (2 DMA loads, 3 I/O
    buffers per core instead of ~31).
"""

import os
import numpy as np

D_MODEL = 64
N_LAYER = 4
D_INNER = 128
D_STATE = 16
D_CONV = 4
DT_RANK = 4
EPS = 1e-5
T = 2048
B = 8
NCORES = 8
MM = 512               # matmul free dim (one PSUM bank)
NCH = T // MM          # matmul chunks
SG = 4                 # states per scan group
NSG = D_STATE // SG    # scan groups
BLK = 2 * T + 1        # per-state scan block: [d0 T | boundary | d1 T]

# ---------------- blob layouts (shared by host prep and device build) ------
def _layout_f32():
    off = {}
    c = 0
    off["x"] = c; c += T                       # [128, T] rows 0:64 d0, 64:128 d1
    off["ll_b"] = c; c += 1                    # [64,1] padded
    off["nfw"] = c; c += 1
    off["nfwm"] = c; c += 1
    off["nfb"] = c; c += 1
    for l in range(N_LAYER):
        off[f"nw{l}"] = c; c += 1
        off[f"nwm{l}"] = c; c += 1             # -nw
        off[f"nb{l}"] = c; c += 1
        off[f"convw{l}"] = c; c += 2 * D_CONV  # [128, d*4+k]
        off[f"convb{l}"] = c; c += 2
        off[f"dtb{l}"] = c; c += 2
        off[f"A{l}"] = c; c += 2 * D_STATE     # [128, d*16+s]
        off[f"D{l}"] = c; c += 2
    return off, c


def _layout_16():
    off = {}
    c = 0
    off["lnsel"] = c; c += 2                   # [128,2] dir-mask/64 stats lhsT
    off["bcast64"] = c; c += 128               # rows 0:2, [2,128] row-bcast lhsT
    off["pool"] = c; c += 2                    # [128,2] pooling lhsT (dir cols)
    off["ll_wT"] = c; c += 64                  # [128,64]
    for l in range(N_LAYER):
        off[f"inw{l}"] = c; c += 2 * D_INNER   # [128,256] rows split by dir
    for d in range(2):
        for l in range(N_LAYER):
            off[f"xproj{d}{l}"] = c; c += 68   # [128,68]
            off[f"dtw{d}{l}"] = c; c += D_INNER  # rows 64:68
            off[f"outw{d}{l}"] = c; c += D_MODEL
    return off, c


OFF_F, NBF = _layout_f32()
OFF_H, NBH = _layout_16()


def _legalize_sync_waits(nc, mybir, maxw=1):
    """walrus accepts one sync-wait command per instruction; split excess
    waits onto preceding same-engine NOPs (semantically identical)."""
    for blk in nc.m.functions[0].blocks:
        newlist, changed = [], False
        for inst in blk.instructions:
            si = inst.sync_info
            waits = list(si.on_wait) if si and si.on_wait else []
            if len(waits) > maxw:
                k = 0
                while len(waits) > maxw:
                    chunk, waits = waits[:maxw], waits[maxw:]
                    newlist.append(mybir.InstNoOp(
                        name=f"{inst.name}-waitsplit{k}", engine=inst.engine,
                        sync_info=mybir.SyncInfo(on_wait=chunk, on_update=[])))
                    k += 1
                inst.sync_info = mybir.SyncInfo(
                    on_wait=waits, on_update=list(si.on_update or []))
                changed = True
            newlist.append(inst)
        if changed:
            blk.instructions = newlist


def build_nc(legalize=True):
    import concourse.bass as bass
    import concourse.mybir as mybir
    import concourse.tile as tile
    from contextlib import ExitStack

    dt32 = mybir.dt.float32
    dt32r = mybir.dt.float32r
    dt16 = mybir.dt.bfloat16
    Alu = mybir.AluOpType
    Act = mybir.ActivationFunctionType
    AP = bass.AP

    nc = bass.Bass("TRN2", target_bir_lowering=False, debug=False,
                   num_devices=NCORES)

    blobf_d = nc.dram_tensor("blobf", [128, NBF], dt32, kind="ExternalInput").ap()
    blobh_d = nc.dram_tensor("blobh", [128, NBH], dt16, kind="ExternalInput").ap()
    scratch = nc.dram_tensor("scratch", [48, 2 * T], dt16, kind="Internal").ap()
    out_d = nc.dram_tensor("out", [D_MODEL, 1], dt32, kind="ExternalOutput").ap()

    n_layers = int(os.environ.get("BK_LAYERS", N_LAYER))
    do_head = os.environ.get("BK_HEAD", "1") == "1"
    n_sg = int(os.environ.get("BK_NSG", NSG))      # scan groups (timing knob)
    no_dma = os.environ.get("BK_NODMA", "0") == "1"  # memset B/C (timing knob)

    def rep(ap_, dims):
        """Raw-AP constructor from the ap's tensor/offset with given
        [stride, n] dims (strides in elements)."""
        return AP(ap_.tensor, ap_.offset, [list(x) for x in dims])

    with tile.TileContext(nc) as tc, ExitStack() as ctx:
        const = ctx.enter_context(tc.tile_pool(name="const", bufs=1))
        sb = ctx.enter_context(tc.tile_pool(name="sb", bufs=2))
        act = ctx.enter_context(tc.tile_pool(name="act", bufs=1))
        scn = ctx.enter_context(tc.tile_pool(name="scn", bufs=1))
        rows = ctx.enter_context(tc.tile_pool(name="rows", bufs=1))
        pp = ctx.enter_context(tc.tile_pool(name="pp", bufs=3, space="PSUM"))
        pbc = ctx.enter_context(tc.tile_pool(name="pbc", bufs=2, space="PSUM"))

        blobf = const.tile([128, NBF], dt32, tag="blobf")
        nc.sync.dma_start(out=blobf, in_=blobf_d)
        blobh = const.tile([128, NBH], dt16, tag="blobh")
        nc.sync.dma_start(out=blobh, in_=blobh_d)
        ones_col = const.tile([128, 1], dt32, tag="ones")
        nc.vector.memset(ones_col, 1.0)
        eps2 = const.tile([2, 1], dt32, tag="eps2")
        nc.vector.memset(eps2, EPS)

        def F(name):  # f32 blob slice helper
            return OFF_F[name]

        ppitch = blobf.ap[0][0]  # sbuf partition pitch in elements (f32)
        hpitch = blobh.ap[0][0]

        # stats+apply layernorm over dir-split rows; returns [128, T] tile
        # hln = ((src - mu) * rstd) * nw + nb ; written in dtype `odt`.
        # mu/rstd/mu*rstd live in separate [2, T] tiles so every matmul
        # operand starts at partition base 0.
        def layer_norm(src, nw_off, nwm_off, nb_off, odt, tag):
            mu_t = rows.tile([2, T], dt16, tag="mu")
            q_t = rows.tile([2, T], dt16, tag="q")
            lnsel = blobh[:, OFF_H["lnsel"]:OFF_H["lnsel"] + 2]
            bc64 = rep(blobh[0:2, OFF_H["bcast64"]:], [[hpitch, 2], [1, 128]])
            for c in range(NCH):
                sj = slice(c * MM, (c + 1) * MM)
                rb16 = sb.tile([128, MM], dt16, tag="rbf")
                nc.scalar.activation(rb16, src[:, sj], Act.Copy)
                xsq = sb.tile([128, MM], dt16, tag="xsq")
                nc.scalar.activation(xsq, src[:, sj], Act.Square)
                pm = pbc.tile([2, MM], dt32, tag="pstat")
                nc.tensor.matmul(pm, lnsel, rb16, start=True, stop=True)
                pq = pbc.tile([2, MM], dt32, tag="pstat")
                nc.tensor.matmul(pq, lnsel, xsq, start=True, stop=True)
                nc.scalar.activation(mu_t[:, sj], pm, Act.Copy)
                nc.scalar.activation(q_t[:, sj], pq, Act.Copy)
            musq = rows.tile([2, T], dt16, tag="mr16")
            nc.scalar.activation(musq, mu_t, Act.Square)
            with nc.allow_low_precision("LN stats in bf16"):
                nc.vector.tensor_sub(q_t, q_t, musq)
                nc.scalar.activation(q_t, q_t, Act.Sqrt, bias=eps2)
                nc.vector.reciprocal(q_t, q_t)      # q_t now holds rstd
                mr_t = rows.tile([2, T], dt16, tag="mr16")
                nc.vector.tensor_mul(mr_t, mu_t, q_t)
            hln = act.tile([128, T], odt, tag=tag)
            nw = blobf[:, nw_off:nw_off + 1]
            nwm = blobf[:, nwm_off:nwm_off + 1]
            for c in range(NCH):
                sj = slice(c * MM, (c + 1) * MM)
                rb = pbc.tile([128, MM], dt32, tag="pb")
                nc.tensor.matmul(rb, bc64, q_t[:, sj], start=True, stop=True)
                mb = pbc.tile([128, MM], dt32, tag="pb")
                nc.tensor.matmul(mb, bc64, mr_t[:, sj], start=True, stop=True)
                t1 = sb.tile([128, MM], dt32, tag="lnt")
                nc.vector.scalar_tensor_tensor(t1, src[:, sj], nw, rb,
                                               op0=Alu.mult, op1=Alu.mult)
                nc.vector.scalar_tensor_tensor(t1, mb, nwm, t1,
                                               op0=Alu.mult, op1=Alu.add)
                nc.scalar.activation(hln[:, sj], t1, Act.Identity,
                                     bias=blobf[:, nb_off:nb_off + 1])
            return hln

        res = blobf[:, F("x"):F("x") + T]   # layer-0 residual = input

        for l in range(n_layers):
            hln = layer_norm(res, F(f"nw{l}"), F(f"nwm{l}"), F(f"nb{l}"),
                             dt16, "hln")

            # ---- in_proj: x -> xpad (dir blocks with 3-col conv head) -----
            # xpad layout: [d0: 3 | 2048][d1: 3 | 2048] = 2*(3+T) cols
            CW = D_CONV - 1 + T
            xpad = act.tile([128, 2 * CW], dt16, tag="xpad")
            nc.vector.memset(
                rep(xpad, [[xpad.ap[0][0], 128], [CW, 2], [1, D_CONV - 1]]), 0.0)
            zsilu = act.tile([128, 2 * T], dt16, tag="zsilu")
            inw = blobh[:, OFF_H[f"inw{l}"]:OFF_H[f"inw{l}"] + 2 * D_INNER]
            for d in range(2):
                rbase = d * D_MODEL
                wl = inw[rbase:rbase + D_MODEL]
                for c in range(NCH):
                    sj = slice(c * MM, (c + 1) * MM)
                    rhs = hln[rbase:rbase + D_MODEL, sj]
                    px = pp.tile([128, MM], dt32, tag="pp")
                    nc.tensor.matmul(px, wl[:, 0:D_INNER], rhs,
                                     start=True, stop=True)
                    o0 = d * CW + D_CONV - 1 + c * MM
                    nc.scalar.activation(xpad[:, o0:o0 + MM], px, Act.Copy)
                    pz = pp.tile([128, MM], dt32, tag="pp")
                    nc.tensor.matmul(pz, wl[:, D_INNER:], rhs,
                                     start=True, stop=True)
                    z0 = d * T + c * MM
                    nc.scalar.activation(zsilu[:, z0:z0 + MM], pz, Act.Sigmoid)
                    with nc.allow_low_precision("silu in bf16"):
                        nc.vector.tensor_mul(zsilu[:, z0:z0 + MM],
                                             zsilu[:, z0:z0 + MM], pz)

            # ---- causal depthwise conv + silu -> xs [128, 2T] -------------
            xs = act.tile([128, 2 * T], dt16, tag="xs")
            cw = blobf[:, F(f"convw{l}"):F(f"convw{l}") + 2 * D_CONV]
            cb = blobf[:, F(f"convb{l}"):F(f"convb{l}") + 2]
            for d in range(2):
                eng = nc.vector
                xv = lambda k: xpad[:, d * CW + k:d * CW + k + T]
                dst = xs[:, d * T:(d + 1) * T]
                eng.tensor_scalar(dst, xv(0), cw[:, 4 * d:4 * d + 1],
                                  cb[:, d:d + 1], op0=Alu.mult, op1=Alu.add)
                for k in range(1, D_CONV):
                    eng.scalar_tensor_tensor(dst, xv(k),
                                             cw[:, 4 * d + k:4 * d + k + 1],
                                             dst, op0=Alu.mult, op1=Alu.add)
            xsig = act.tile([128, 2 * T], dt16, tag="xsig")
            nc.scalar.activation(xsig, xs, Act.Sigmoid)
            with nc.allow_low_precision("silu in bf16"):
                nc.gpsimd.tensor_mul(xs, xs, xsig)

            # ---- xproj -> bcs [68, 2T] (B 0:16, C 32:48, dtraw 64:68) -----
            bcs = act.tile([68, 2 * T], dt16, tag="bcs")
            for d in range(2):
                xw = blobh[:, OFF_H[f"xproj{d}{l}"]:OFF_H[f"xproj{d}{l}"] + 68]
                for c in range(NCH):
                    pd_ = pp.tile([68, MM], dt32, tag="pp")
                    nc.tensor.matmul(pd_, xw,
                                     xs[:, d * T + c * MM:d * T + (c + 1) * MM],
                                     start=True, stop=True)
                    nc.scalar.activation(bcs[:, d * T + c * MM:
                                              d * T + (c + 1) * MM],
                                         pd_, Act.Copy)
            nc.sync.dma_start(out=scratch, in_=bcs[0:48, :])

            # ---- dt = softplus(dt_w @ dtraw + dt_b) -> dts bf16 [128,2T] --
            dts = act.tile([128, 2 * T], dt16, tag="dts")
            dtb = blobf[:, F(f"dtb{l}"):F(f"dtb{l}") + 2]
            for d in range(2):
                dw = blobh[:, OFF_H[f"dtw{d}{l}"]:OFF_H[f"dtw{d}{l}"] + D_INNER]
                for c in range(NCH):
                    pt = pp.tile([128, MM], dt32, tag="pp")
                    nc.tensor.matmul(pt, dw[64:68, :],
                                     bcs[64:68, d * T + c * MM:
                                         d * T + (c + 1) * MM],
                                     start=True, stop=True)
                    nc.scalar.activation(dts[:, d * T + c * MM:
                                              d * T + (c + 1) * MM],
                                         pt, Act.Exp, bias=dtb[:, d:d + 1])
            nc.scalar.activation(dts, dts, Act.Ln, bias=ones_col)

            # ---- u = dts * xs --------------------------------------------
            u = act.tile([128, 2 * T], dt16, tag="xpad")
            nc.gpsimd.tensor_mul(u, dts, xs)

            # ---- selective scan, 4 states per group ----------------------
            yacc = act.tile([128, 2 * T], dt16, tag="bcs")
            if n_sg == 0:
                nc.vector.memset(yacc, 0.0)
            A2 = blobf[:, F(f"A{l}"):F(f"A{l}") + 2 * D_STATE]
            for g in range(n_sg):
                dbx = scn.tile([128, SG * BLK], dt16, tag="dbx",
                               name=f"dbx{l}_{g}")
                da = scn.tile([128, SG * BLK], dt16, tag="da",
                              name=f"da{l}_{g}")
                dpit = dbx.ap[0][0]
                # B broadcast from DRAM: dbx[p, s*BLK + d*(T+1) + t] = B[g4+s, d*T+t]
                # (one DMA per direction so both APs optimize to <=3 dims)
                dst3 = rep(dbx, [[dpit, 128], [BLK, SG], [T + 1, 2], [1, T]])
                if no_dma:
                    nc.gpsimd.memset(dst3, 0.01)
                else:
                    for d in range(2):
                        dstd = AP(dbx.tensor, dbx.offset + d * (T + 1),
                                  [[dpit, 128], [BLK, SG], [1, T]])
                        srcd = AP(scratch.tensor,
                                  scratch.offset + (g * SG) * (2 * T) + d * T,
                                  [[0, 128], [2 * T, SG], [1, T]])
                        (nc.sync if d == 0 else nc.scalar).dma_start(
                            out=dstd, in_=srcd)
                # dbx = u(rep states) * Bb  (in place over the dma data)
                u3 = rep(u, [[u.ap[0][0], 128], [0, SG], [T, 2], [1, T]])
                with nc.allow_low_precision("dbx in bf16"):
                    nc.vector.tensor_tensor(dst3, u3, dst3, op=Alu.mult)
                # zero the boundary columns
                nc.vector.memset(
                    rep(AP(dbx.tensor, dbx.offset + T, dbx.ap),
                        [[dpit, 128], [BLK, SG]]), 0.0)
                # da = exp(dts * A) with boundary zeros
                da3 = rep(da, [[dpit, 128], [BLK, SG], [T + 1, 2], [1, T]])
                dts3 = rep(dts, [[dts.ap[0][0], 128], [0, SG], [T, 2], [1, T]])
                A3 = rep(AP(blobf.tensor, A2.offset + g * SG, A2.ap),
                         [[ppitch, 128], [1, SG], [D_STATE, 2], [0, T]])
                with nc.allow_low_precision("da in bf16"):
                    nc.vector.tensor_tensor(da3, dts3, A3, op=Alu.mult)
                nc.vector.memset(
                    rep(AP(da.tensor, da.offset + T, da.ap),
                        [[dpit, 128], [BLK, SG]]), 0.0)
                nc.scalar.activation(da3, da3, Act.Exp)
                # scans (hs overwrites da)
                for s in range(SG):
                    with nc.allow_low_precision("scan in bf16"):
                        nc.vector.tensor_tensor_scan(
                            da[:, s * BLK:(s + 1) * BLK],
                            da[:, s * BLK:(s + 1) * BLK],
                            dbx[:, s * BLK:(s + 1) * BLK],
                            0.0, op0=Alu.mult, op1=Alu.add)
                # C broadcast into its own tile (overlaps the scans);
                # y = hs * Cb written back over hs (in the da tile)
                cb = scn.tile([128, SG * BLK], dt16, tag="cb",
                              name=f"cb{l}_{g}")
                nc.vector.memset(
                    rep(AP(cb.tensor, cb.offset + T, cb.ap),
                        [[dpit, 128], [BLK, SG]]), 0.0)
                if no_dma:
                    nc.gpsimd.memset(
                        rep(cb, [[dpit, 128], [BLK, SG], [T + 1, 2], [1, T]]),
                        0.01)
                else:
                    for d in range(2):
                        dstd = AP(cb.tensor, cb.offset + d * (T + 1),
                                  [[dpit, 128], [BLK, SG], [1, T]])
                        srcd = AP(scratch.tensor,
                                  scratch.offset + (32 + g * SG) * (2 * T) + d * T,
                                  [[0, 128], [2 * T, SG], [1, T]])
                        (nc.sync if d == 1 else nc.scalar).dma_start(
                            out=dstd, in_=srcd)
                with nc.allow_low_precision("y in bf16"):
                    nc.vector.tensor_mul(da, da, cb)
                    # reduce 4 state planes -> 2 -> 1
                    nc.vector.tensor_add(
                        rep(da, [[dpit, 128], [BLK, 2], [1, BLK]]),
                        rep(da, [[dpit, 128], [BLK, 2], [1, BLK]]),
                        rep(AP(da.tensor, da.offset + 2 * BLK, da.ap),
                            [[dpit, 128], [BLK, 2], [1, BLK]]))
                y3 = rep(yacc, [[yacc.ap[0][0], 128], [T, 2], [1, T]])
                b0 = rep(da, [[dpit, 128], [T + 1, 2], [1, T]])
                b1 = rep(AP(da.tensor, da.offset + BLK, da.ap),
                         [[dpit, 128], [T + 1, 2], [1, T]])
                if g == 0:
                    nc.vector.tensor_add(y3, b0, b1)
                else:
                    nc.vector.tensor_add(b0, b0, b1)
                    nc.vector.tensor_add(y3, y3, b0)

            # ---- y = (xs*D + yacc) * zsilu ; out_proj + residual ----------
            Drep = rep(blobf[:, F(f"D{l}"):], [[ppitch, 128], [1, 2], [0, T]])
            xs3 = rep(xs, [[xs.ap[0][0], 128], [T, 2], [1, T]])
            ya3 = rep(yacc, [[yacc.ap[0][0], 128], [T, 2], [1, T]])
            yf = act.tile([128, 2 * T], dt16, tag="xsig")
            yf3 = rep(yf, [[yf.ap[0][0], 128], [T, 2], [1, T]])
            nc.gpsimd.tensor_tensor(yf3, xs3, Drep, op=Alu.mult)
            with nc.allow_low_precision("y in bf16"):
                nc.vector.tensor_add(yf, yf, yacc)
                nc.gpsimd.tensor_mul(yf, yf, zsilu)
            res_new = sb.tile([128, T], dt32, tag="res", name=f"res{l}")
            for d in range(2):
                ow = blobh[:, OFF_H[f"outw{d}{l}"]:OFF_H[f"outw{d}{l}"] + D_MODEL]
                for c in range(NCH):
                    sj = slice(c * MM, (c + 1) * MM)
                    po = pp.tile([D_MODEL, MM], dt32, tag="pp")
                    nc.tensor.matmul(po, ow, yf[:, d * T + c * MM:
                                                d * T + (c + 1) * MM],
                                     start=True, stop=True)
                    nc.vector.tensor_add(
                        res_new[d * D_MODEL:(d + 1) * D_MODEL, sj], po,
                        res[d * D_MODEL:(d + 1) * D_MODEL, sj])
            res = res_new

        # ---- head: final LN, softmax pool, linear -------------------------
        if do_head:
            hf = layer_norm(res, F("nfw"), F("nfwm"), F("nfb"), dt16, "xpad")
            logits = rows.tile([2, T], dt16, tag="logits")
            plhs = blobh[:, OFF_H["pool"]:OFF_H["pool"] + 2]
            for c in range(NCH):
                sj = slice(c * MM, (c + 1) * MM)
                pl = pbc.tile([2, MM], dt32, tag="pstat")
                nc.tensor.matmul(pl, plhs, hf[:, sj], start=True, stop=True)
                nc.scalar.activation(logits[:, sj], pl, Act.Copy)
            sm = rows.tile([2, 4], dt32, tag="sm")
            nc.vector.reduce_max(sm[:, 0:1], logits, axis=mybir.AxisListType.X)
            nc.vector.tensor_scalar_mul(sm[:, 1:2], sm[:, 0:1], -1.0)
            nc.scalar.activation(logits, logits, Act.Exp, bias=sm[:, 1:2])
            nc.vector.reduce_sum(sm[:, 2:3], logits, axis=mybir.AxisListType.X)
            nc.vector.reciprocal(sm[:, 3:4], sm[:, 2:3])
            with nc.allow_low_precision("softmax weights bf16"):
                nc.vector.tensor_scalar_mul(logits, logits, sm[:, 3:4])
            bc64 = rep(blobh[0:2, OFF_H["bcast64"]:], [[hpitch, 2], [1, 128]])
            for c in range(NCH):
                sj = slice(c * MM, (c + 1) * MM)
                ab = pbc.tile([128, MM], dt32, tag="pb")
                nc.tensor.matmul(ab, bc64, logits[:, sj], start=True, stop=True)
                with nc.allow_low_precision("pool in bf16"):
                    nc.vector.tensor_mul(hf[:, sj], hf[:, sj], ab)
            pooled = rows.tile([128, 1], dt16, tag="pooled")
            with nc.allow_low_precision("pool in bf16"):
                nc.vector.reduce_sum(pooled, hf, axis=mybir.AxisListType.X)
            pout = pp.tile([D_MODEL, 1], dt32, tag="pp")
            nc.tensor.matmul(pout,
                             blobh[:, OFF_H["ll_wT"]:OFF_H["ll_wT"] + 64],
                             pooled, start=True, stop=True)
            out_sb = rows.tile([D_MODEL, 1], dt32, tag="outsb")
            nc.scalar.activation(out_sb, pout, Act.Identity,
                                 bias=blobf[0:64, F("ll_b"):F("ll_b") + 1])
            nc.sync.dma_start(out=out_d, in_=out_sb)
        else:
            out_sb = rows.tile([D_MODEL, 1], dt32, tag="outsb")
            nc.vector.tensor_copy(out_sb, res[0:D_MODEL, 0:1])
            nc.sync.dma_start(out=out_d, in_=out_sb)

    if legalize:
        _legalize_sync_waits(nc, mybir)
    return nc


def prep_inputs(inputs):
    import ml_dtypes
    f = np.float32
    x = np.asarray(inputs["x"], f).reshape(B, D_MODEL, T)
    xb = x[:, :, ::-1]

    blobf = np.zeros((128, NBF), f)
    O = OFF_F
    blobf[0:64, O["ll_b"]] = np.asarray(inputs["ll_b"], f)
    blobf[0:64, O["nfw"]] = np.asarray(inputs["nf_w"], f)
    blobf[64:128, O["nfw"]] = np.asarray(inputs["nf_w"], f)
    blobf[:, O["nfwm"]] = -blobf[:, O["nfw"]]
    blobf[0:64, O["nfb"]] = np.asarray(inputs["nf_b"], f)
    blobf[64:128, O["nfb"]] = np.asarray(inputs["nf_b"], f)
    nw, nb = np.asarray(inputs["nw"], f), np.asarray(inputs["nb"], f)
    conv_w = np.asarray(inputs["conv_w"], f)
    conv_b = np.asarray(inputs["conv_b"], f)
    dt_b = np.asarray(inputs["dt_b"], f)
    A = -np.exp(np.asarray(inputs["A_log"], f))       # [2,4,128,16]
    Dp = np.asarray(inputs["D"], f)
    for l in range(N_LAYER):
        blobf[0:64, O[f"nw{l}"]] = nw[0, l]
        blobf[64:128, O[f"nw{l}"]] = nw[1, l]
        blobf[:, O[f"nwm{l}"]] = -blobf[:, O[f"nw{l}"]]
        blobf[0:64, O[f"nb{l}"]] = nb[0, l]
        blobf[64:128, O[f"nb{l}"]] = nb[1, l]
        for d in range(2):
            blobf[:, O[f"convw{l}"] + 4 * d:O[f"convw{l}"] + 4 * d + 4] = \
                conv_w[d, l]
            blobf[:, O[f"convb{l}"] + d] = conv_b[d, l]
            blobf[:, O[f"dtb{l}"] + d] = dt_b[d, l]
            blobf[:, O[f"A{l}"] + 16 * d:O[f"A{l}"] + 16 * d + 16] = A[d, l]
            blobf[:, O[f"D{l}"] + d] = Dp[d, l]

    blobh = np.zeros((128, NBH), f)
    H = OFF_H
    blobh[0:64, H["lnsel"]] = 1.0 / D_MODEL
    blobh[64:128, H["lnsel"] + 1] = 1.0 / D_MODEL
    blobh[0, H["bcast64"]:H["bcast64"] + 64] = 1.0
    blobh[1, H["bcast64"] + 64:H["bcast64"] + 128] = 1.0
    blobh[0:64, H["pool"]] = np.asarray(inputs["fp_w"], f)[0]
    blobh[64:128, H["pool"] + 1] = np.asarray(inputs["bp_w"], f)[0]
    blobh[:, H["ll_wT"]:H["ll_wT"] + 64] = np.asarray(inputs["ll_w"], f).T
    in_w = np.asarray(inputs["in_w"], f)              # [2,4,256,64]
    xproj_w = np.asarray(inputs["xproj_w"], f)        # [2,4,36,128]
    dt_w = np.asarray(inputs["dt_w"], f)              # [2,4,128,4]
    out_w = np.asarray(inputs["out_w"], f)            # [2,4,64,128]
    for l in range(N_LAYER):
        blobh[0:64, H[f"inw{l}"]:H[f"inw{l}"] + 256] = in_w[0, l].T
        blobh[64:128, H[f"inw{l}"]:H[f"inw{l}"] + 256] = in_w[1, l].T
        for d in range(2):
            xp = xproj_w[d, l].T                      # [128, 36]
            blobh[:, H[f"xproj{d}{l}"]:H[f"xproj{d}{l}"] + 16] = \
                xp[:, DT_RANK:DT_RANK + D_STATE]
            blobh[:, H[f"xproj{d}{l}"] + 32:H[f"xproj{d}{l}"] + 48] = \
                xp[:, DT_RANK + D_STATE:]
            blobh[:, H[f"xproj{d}{l}"] + 64:H[f"xproj{d}{l}"] + 68] = \
                xp[:, 0:DT_RANK]
            blobh[64:68, H[f"dtw{d}{l}"]:H[f"dtw{d}{l}"] + D_INNER] = \
                dt_w[d, l].T
            blobh[:, H[f"outw{d}{l}"]:H[f"outw{d}{l}"] + D_MODEL] = \
                out_w[d, l].T
    blobh16 = blobh.astype(ml_dtypes.bfloat16)

    in_maps = []
    for b in range(B):
        bf = blobf.copy()
        bf[0:64, 0:T] = x[b]
        bf[64:128, 0:T] = xb[b]
        in_maps.append({"blobf": bf, "blobh": blobh16})
    return in_maps


def kernel(**inputs):
    from concourse.bass_utils import run_bass_kernel_spmd
    in_maps = prep_inputs(inputs)
    nc = build_nc()
    res = run_bass_kernel_spmd(nc, in_maps, core_ids=list(range(NCORES)))
    out = np.stack([res.results[b]["out"][:, 0] for b in range(B)])
    return out.astype(np.float32)


# revision 9
# speedup vs baseline: 1.0078x; 1.0078x over previous
"""Bidirectional Mamba (MixerModel) Trainium2 kernel, v2.

Sharding: data-parallel over batch - 8 batch elements -> 8 NeuronCores; each
core runs the full 2-direction x 4-layer model for its batch element. The
backward direction consumes a host-flipped input; the softmax pool is
order-invariant so nothing is unflipped. Host stacks the per-core [64] rows.

v2 is an instruction-count rewrite of the working v1 (7128 -> ~1.2k instrs):
  * Both directions live in one tile: rows 0:64 / 64:128 for d_model stages,
    free-axis blocks [dir0 | dir1] for d_inner stages, so every elementwise
    instruction uses all 128 lanes and covers both directions at once.
  * Selective scan runs at full T with a boundary-column trick: each state's
    lane is [dir0(2048) | 1 zeroed col | dir1(2048)], so one
    tensor_tensor_scan instruction covers both directions (the zero column
    resets the recurrence exactly) - 16 scans/layer total, no chunk carries.
  * B/C broadcasts to 128 partitions go through a DRAM round trip: bcs rows
    are written once per layer, then one 4-dim DMA per 4-state group reads
    them back replicated across partitions (SBUF src APs cannot have a
    0-stride partition dim; DRAM src APs can).
  * dbx/da for all states of a group are computed by single wide DVE ops
    using stride-0 repeat APs (u repeated over states, A repeated over time).
  * Depthwise conv = 4 scalar_tensor_tensor FMAs per direction.
  * All matmuls run in bf16 (1 cycle/row on the PE).
  * All inputs packed into one f32 + one bf16 blob (2 DMA loads, 3 I/O
    buffers per core instead of ~31).
"""

import os
import numpy as np

D_MODEL = 64
N_LAYER = 4
D_INNER = 128
D_STATE = 16
D_CONV = 4
DT_RANK = 4
EPS = 1e-5
T = 2048
B = 8
NCORES = 8
MM = 512               # matmul free dim (one PSUM bank)
NCH = T // MM          # matmul chunks
SG = int(__import__("os").environ.get("BK_SG", "4"))  # states per scan group
NSG = D_STATE // SG    # scan groups
BLK = 2 * T + 1        # per-state scan block: [d0 T | boundary | d1 T]

# ---------------- blob layouts (shared by host prep and device build) ------
def _layout_f32():
    off = {}
    c = 0
    off["x"] = c; c += T                       # [128, T] rows 0:64 d0, 64:128 d1
    off["ll_b"] = c; c += 1                    # [64,1] padded
    off["nfw"] = c; c += 1
    off["nfwm"] = c; c += 1
    off["nfb"] = c; c += 1
    for l in range(N_LAYER):
        off[f"nw{l}"] = c; c += 1
        off[f"nwm{l}"] = c; c += 1             # -nw
        off[f"nb{l}"] = c; c += 1
        off[f"convw{l}"] = c; c += 2 * D_CONV  # [128, d*4+k]
        off[f"convb{l}"] = c; c += 2
        off[f"dtb{l}"] = c; c += 2
        off[f"A{l}"] = c; c += 2 * D_STATE     # [128, d*16+s]
        off[f"D{l}"] = c; c += 2
    return off, c


def _layout_16():
    off = {}
    c = 0
    off["lnsel"] = c; c += 2                   # [128,2] dir-mask/64 stats lhsT
    off["bcast64"] = c; c += 128               # rows 0:2, [2,128] row-bcast lhsT
    off["pool"] = c; c += 2                    # [128,2] pooling lhsT (dir cols)
    off["ll_wT"] = c; c += 64                  # [128,64]
    for l in range(N_LAYER):
        off[f"A16{l}"] = c; c += 2 * D_STATE   # [128, d*16+s] (bf16 copy of A)
    for l in range(N_LAYER):
        off[f"inw{l}"] = c; c += 2 * D_INNER   # [128,256] rows split by dir
    for d in range(2):
        for l in range(N_LAYER):
            off[f"xproj{d}{l}"] = c; c += 68   # [128,68]
            off[f"dtw{d}{l}"] = c; c += D_INNER  # rows 64:68
            off[f"outw{d}{l}"] = c; c += D_MODEL
    return off, c


OFF_F, NBF = _layout_f32()
OFF_H, NBH = _layout_16()


def _legalize_sync_waits(nc, mybir, maxw=1):
    """walrus accepts one sync-wait command per instruction; split excess
    waits onto preceding same-engine NOPs (semantically identical)."""
    for blk in nc.m.functions[0].blocks:
        newlist, changed = [], False
        for inst in blk.instructions:
            si = inst.sync_info
            waits = list(si.on_wait) if si and si.on_wait else []
            if len(waits) > maxw:
                k = 0
                while len(waits) > maxw:
                    chunk, waits = waits[:maxw], waits[maxw:]
                    newlist.append(mybir.InstNoOp(
                        name=f"{inst.name}-waitsplit{k}", engine=inst.engine,
                        sync_info=mybir.SyncInfo(on_wait=chunk, on_update=[])))
                    k += 1
                inst.sync_info = mybir.SyncInfo(
                    on_wait=waits, on_update=list(si.on_update or []))
                changed = True
            newlist.append(inst)
        if changed:
            blk.instructions = newlist


def build_nc(legalize=True):
    import concourse.bass as bass
    import concourse.mybir as mybir
    import concourse.tile as tile
    from contextlib import ExitStack

    dt32 = mybir.dt.float32
    dt32r = mybir.dt.float32r
    dt16 = mybir.dt.bfloat16
    Alu = mybir.AluOpType
    Act = mybir.ActivationFunctionType
    AP = bass.AP

    nc = bass.Bass("TRN2", target_bir_lowering=False, debug=False,
                   num_devices=NCORES)

    blobf_d = nc.dram_tensor("blobf", [128, NBF], dt32, kind="ExternalInput").ap()
    blobh_d = nc.dram_tensor("blobh", [128, NBH], dt16, kind="ExternalInput").ap()
    scratch = nc.dram_tensor("scratch", [48, 2 * T], dt16, kind="Internal").ap()
    out_d = nc.dram_tensor("out", [D_MODEL, 1], dt32, kind="ExternalOutput").ap()

    n_layers = int(os.environ.get("BK_LAYERS", N_LAYER))
    do_head = os.environ.get("BK_HEAD", "1") == "1"
    n_sg = int(os.environ.get("BK_NSG", NSG))      # scan groups (timing knob)
    no_dma = os.environ.get("BK_NODMA", "0") == "1"  # memset B/C (timing knob)

    def rep(ap_, dims):
        """Raw-AP constructor from the ap's tensor/offset with given
        [stride, n] dims (strides in elements)."""
        return AP(ap_.tensor, ap_.offset, [list(x) for x in dims])

    with tile.TileContext(nc) as tc, ExitStack() as ctx:
        const = ctx.enter_context(tc.tile_pool(name="const", bufs=1))
        sb = ctx.enter_context(tc.tile_pool(name="sb", bufs=2))
        act = ctx.enter_context(tc.tile_pool(name="act", bufs=1))
        scn = ctx.enter_context(tc.tile_pool(name="scn", bufs=1))
        scnb = ctx.enter_context(tc.tile_pool(
            name="scnb", bufs=int(os.environ.get("BK_DBXBUFS", "1"))))
        rows = ctx.enter_context(tc.tile_pool(name="rows", bufs=1))
        pp = ctx.enter_context(tc.tile_pool(name="pp", bufs=3, space="PSUM"))
        pbc = ctx.enter_context(tc.tile_pool(name="pbc", bufs=2, space="PSUM"))

        blobf = const.tile([128, NBF], dt32, tag="blobf")
        nc.sync.dma_start(out=blobf, in_=blobf_d)
        blobh = const.tile([128, NBH], dt16, tag="blobh")
        nc.sync.dma_start(out=blobh, in_=blobh_d)
        ones_col = const.tile([128, 1], dt32, tag="ones")
        nc.vector.memset(ones_col, 1.0)
        eps2 = const.tile([2, 1], dt32, tag="eps2")
        nc.vector.memset(eps2, EPS)

        def F(name):  # f32 blob slice helper
            return OFF_F[name]

        ppitch = blobf.ap[0][0]  # sbuf partition pitch in elements (f32)
        hpitch = blobh.ap[0][0]

        # stats+apply layernorm over dir-split rows; returns [128, T] tile
        # hln = ((src - mu) * rstd) * nw + nb ; written in dtype `odt`.
        # mu/rstd/mu*rstd live in separate [2, T] tiles so every matmul
        # operand starts at partition base 0.
        def layer_norm(src, nw_off, nwm_off, nb_off, odt, tag):
            mu_t = rows.tile([2, T], dt16, tag="mu")
            q_t = rows.tile([2, T], dt16, tag="q")
            lnsel = blobh[:, OFF_H["lnsel"]:OFF_H["lnsel"] + 2]
            bc64 = rep(blobh[0:2, OFF_H["bcast64"]:], [[hpitch, 2], [1, 128]])
            for c in range(NCH):
                sj = slice(c * MM, (c + 1) * MM)
                if src.dtype == dt16:
                    rb16 = src[:, sj]
                else:
                    rb16 = sb.tile([128, MM], dt16, tag="rbf")
                    nc.scalar.activation(rb16, src[:, sj], Act.Copy)
                xsq = sb.tile([128, MM], dt16, tag="xsq")
                nc.scalar.activation(xsq, src[:, sj], Act.Square)
                pm = pbc.tile([2, MM], dt32, tag="pstat")
                nc.tensor.matmul(pm, lnsel, rb16, start=True, stop=True)
                pq = pbc.tile([2, MM], dt32, tag="pstat")
                nc.tensor.matmul(pq, lnsel, xsq, start=True, stop=True)
                nc.scalar.activation(mu_t[:, sj], pm, Act.Copy)
                nc.scalar.activation(q_t[:, sj], pq, Act.Copy)
            musq = rows.tile([2, T], dt16, tag="mr16")
            nc.scalar.activation(musq, mu_t, Act.Square)
            with nc.allow_low_precision("LN stats in bf16"):
                nc.vector.tensor_sub(q_t, q_t, musq)
                nc.scalar.activation(q_t, q_t, Act.Sqrt, bias=eps2)
                nc.vector.reciprocal(q_t, q_t)      # q_t now holds rstd
                mr_t = rows.tile([2, T], dt16, tag="mr16")
                nc.vector.tensor_mul(mr_t, mu_t, q_t)
            hln = act.tile([128, T], odt, tag=tag)
            nw = blobf[:, nw_off:nw_off + 1]
            nwm = blobf[:, nwm_off:nwm_off + 1]
            for c in range(NCH):
                sj = slice(c * MM, (c + 1) * MM)
                rb = pbc.tile([128, MM], dt32, tag="pb")
                nc.tensor.matmul(rb, bc64, q_t[:, sj], start=True, stop=True)
                mb = pbc.tile([128, MM], dt32, tag="pb")
                nc.tensor.matmul(mb, bc64, mr_t[:, sj], start=True, stop=True)
                t1 = sb.tile([128, MM], dt32, tag="lnt")
                nc.vector.scalar_tensor_tensor(t1, src[:, sj], nw, rb,
                                               op0=Alu.mult, op1=Alu.mult)
                nc.vector.scalar_tensor_tensor(t1, mb, nwm, t1,
                                               op0=Alu.mult, op1=Alu.add)
                nc.scalar.activation(hln[:, sj], t1, Act.Identity,
                                     bias=blobf[:, nb_off:nb_off + 1])
            return hln

        res = blobf[:, F("x"):F("x") + T]   # layer-0 residual = input

        for l in range(n_layers):
            hln = layer_norm(res, F(f"nw{l}"), F(f"nwm{l}"), F(f"nb{l}"),
                             dt16, "dts")

            # ---- in_proj: x -> xpad (dir blocks with 3-col conv head) -----
            # xpad layout: [d0: 3 | 2048][d1: 3 | 2048] = 2*(3+T) cols
            CW = D_CONV - 1 + T
            xpad = act.tile([128, 2 * CW], dt16, tag="xpad")
            nc.vector.memset(
                rep(xpad, [[xpad.ap[0][0], 128], [CW, 2], [1, D_CONV - 1]]), 0.0)
            zsilu = act.tile([128, 2 * T], dt16, tag="zsilu")
            inw = blobh[:, OFF_H[f"inw{l}"]:OFF_H[f"inw{l}"] + 2 * D_INNER]
            for d in range(2):
                rbase = d * D_MODEL
                wl = inw[rbase:rbase + D_MODEL]
                for c in range(NCH):
                    sj = slice(c * MM, (c + 1) * MM)
                    rhs = hln[rbase:rbase + D_MODEL, sj]
                    px = pp.tile([128, MM], dt32, tag="pp")
                    nc.tensor.matmul(px, wl[:, 0:D_INNER], rhs,
                                     start=True, stop=True)
                    o0 = d * CW + D_CONV - 1 + c * MM
                    nc.scalar.activation(xpad[:, o0:o0 + MM], px, Act.Copy)
                    pz = pp.tile([128, MM], dt32, tag="pp")
                    nc.tensor.matmul(pz, wl[:, D_INNER:], rhs,
                                     start=True, stop=True)
                    z0 = d * T + c * MM
                    nc.scalar.activation(zsilu[:, z0:z0 + MM], pz, Act.Sigmoid)
                    with nc.allow_low_precision("silu in bf16"):
                        nc.vector.tensor_mul(zsilu[:, z0:z0 + MM],
                                             zsilu[:, z0:z0 + MM], pz)

            # ---- causal depthwise conv + silu -> xs [128, 2T] -------------
            xs = act.tile([128, 2 * T], dt16, tag="xs")
            cw = blobf[:, F(f"convw{l}"):F(f"convw{l}") + 2 * D_CONV]
            cb = blobf[:, F(f"convb{l}"):F(f"convb{l}") + 2]
            for d in range(2):
                eng = nc.vector
                xv = lambda k: xpad[:, d * CW + k:d * CW + k + T]
                dst = xs[:, d * T:(d + 1) * T]
                eng.tensor_scalar(dst, xv(0), cw[:, 4 * d:4 * d + 1],
                                  cb[:, d:d + 1], op0=Alu.mult, op1=Alu.add)
                for k in range(1, D_CONV):
                    eng.scalar_tensor_tensor(dst, xv(k),
                                             cw[:, 4 * d + k:4 * d + k + 1],
                                             dst, op0=Alu.mult, op1=Alu.add)
            xsig = act.tile([128, 2 * T], dt16, tag="xsig")
            nc.scalar.activation(xsig, xs, Act.Sigmoid)
            with nc.allow_low_precision("silu in bf16"):
                nc.gpsimd.tensor_mul(xs, xs, xsig)

            # ---- xproj -> bcs [68, 2T] (B 0:16, C 32:48, dtraw 64:68) -----
            bcs = act.tile([68, 2 * T], dt16, tag="bcs")
            for d in range(2):
                xw = blobh[:, OFF_H[f"xproj{d}{l}"]:OFF_H[f"xproj{d}{l}"] + 68]
                for c in range(NCH):
                    pd_ = pp.tile([68, MM], dt32, tag="pp")
                    nc.tensor.matmul(pd_, xw,
                                     xs[:, d * T + c * MM:d * T + (c + 1) * MM],
                                     start=True, stop=True)
                    nc.scalar.activation(bcs[:, d * T + c * MM:
                                              d * T + (c + 1) * MM],
                                         pd_, Act.Copy)
            nc.sync.dma_start(out=scratch, in_=bcs[0:48, :])

            # ---- dt = softplus(dt_w @ dtraw + dt_b) -> dts bf16 [128,2T] --
            dts = act.tile([128, 2 * T], dt16, tag="dts")
            dtb = blobf[:, F(f"dtb{l}"):F(f"dtb{l}") + 2]
            for d in range(2):
                dw = blobh[:, OFF_H[f"dtw{d}{l}"]:OFF_H[f"dtw{d}{l}"] + D_INNER]
                for c in range(NCH):
                    pt = pp.tile([128, MM], dt32, tag="pp")
                    nc.tensor.matmul(pt, dw[64:68, :],
                                     bcs[64:68, d * T + c * MM:
                                         d * T + (c + 1) * MM],
                                     start=True, stop=True)
                    nc.scalar.activation(dts[:, d * T + c * MM:
                                              d * T + (c + 1) * MM],
                                         pt, Act.Exp, bias=dtb[:, d:d + 1])
            nc.scalar.activation(dts, dts, Act.Ln, bias=ones_col)

            # ---- u = dts * xs --------------------------------------------
            u = act.tile([128, 2 * T], dt16, tag="xpad")
            nc.gpsimd.tensor_mul(u, dts, xs)

            # ---- selective scan, 4 states per group ----------------------
            yacc = act.tile([128, 2 * T], dt16, tag="bcs")
            if n_sg == 0:
                nc.vector.memset(yacc, 0.0)
            A2 = blobh[:, OFF_H[f"A16{l}"]:OFF_H[f"A16{l}"] + 2 * D_STATE]
            for g in range(n_sg):
                dbx = scnb.tile([128, SG * BLK], dt16, tag="dbx",
                               name=f"dbx{l}_{g}")
                da = scn.tile([128, SG * BLK], dt16, tag="da",
                              name=f"da{l}_{g}")
                dpit = dbx.ap[0][0]
                # B broadcast from DRAM: dbx[p, s*BLK + d*(T+1) + t] = B[g4+s, d*T+t]
                # (one DMA per direction so both APs optimize to <=3 dims)
                dst3 = rep(dbx, [[dpit, 128], [BLK, SG], [T + 1, 2], [1, T]])
                if no_dma:
                    nc.gpsimd.memset(dst3, 0.01)
                else:
                    for d in range(2):
                        dstd = AP(dbx.tensor, dbx.offset + d * (T + 1),
                                  [[dpit, 128], [BLK, SG], [1, T]])
                        srcd = AP(scratch.tensor,
                                  scratch.offset + (g * SG) * (2 * T) + d * T,
                                  [[0, 128], [2 * T, SG], [1, T]])
                        (nc.sync if d == 0 else nc.scalar).dma_start(
                            out=dstd, in_=srcd)
                # dbx = u(rep states) * Bb  (in place over the dma data)
                u3 = rep(u, [[u.ap[0][0], 128], [0, SG], [T, 2], [1, T]])
                with nc.allow_low_precision("dbx in bf16"):
                    nc.vector.tensor_tensor(dst3, u3, dst3, op=Alu.mult)
                # zero the boundary columns
                nc.vector.memset(
                    rep(AP(dbx.tensor, dbx.offset + T, dbx.ap),
                        [[dpit, 128], [BLK, SG]]), 0.0)
                # da = exp(dts * A) with boundary zeros
                da3 = rep(da, [[dpit, 128], [BLK, SG], [T + 1, 2], [1, T]])
                dts3 = rep(dts, [[dts.ap[0][0], 128], [0, SG], [T, 2], [1, T]])
                A3 = rep(AP(blobh.tensor, A2.offset + g * SG, A2.ap),
                         [[hpitch, 128], [1, SG], [D_STATE, 2], [0, T]])
                with nc.allow_low_precision("da in bf16"):
                    nc.vector.tensor_tensor(da3, dts3, A3, op=Alu.mult)
                nc.vector.memset(
                    rep(AP(da.tensor, da.offset + T, da.ap),
                        [[dpit, 128], [BLK, SG]]), 0.0)
                nc.scalar.activation(da3, da3, Act.Exp)
                # scans (hs overwrites da)
                for s in range(SG):
                    with nc.allow_low_precision("scan in bf16"):
                        nc.vector.tensor_tensor_scan(
                            da[:, s * BLK:(s + 1) * BLK],
                            da[:, s * BLK:(s + 1) * BLK],
                            dbx[:, s * BLK:(s + 1) * BLK],
                            0.0, op0=Alu.mult, op1=Alu.add)
                # C broadcast into its own tile (overlaps the scans);
                # y = hs * Cb written back over hs (in the da tile)
                cb = scn.tile([128, SG * BLK], dt16, tag="cb",
                              name=f"cb{l}_{g}")
                nc.vector.memset(
                    rep(AP(cb.tensor, cb.offset + T, cb.ap),
                        [[dpit, 128], [BLK, SG]]), 0.0)
                if no_dma:
                    nc.gpsimd.memset(
                        rep(cb, [[dpit, 128], [BLK, SG], [T + 1, 2], [1, T]]),
                        0.01)
                else:
                    for d in range(2):
                        dstd = AP(cb.tensor, cb.offset + d * (T + 1),
                                  [[dpit, 128], [BLK, SG], [1, T]])
                        srcd = AP(scratch.tensor,
                                  scratch.offset + (32 + g * SG) * (2 * T) + d * T,
                                  [[0, 128], [2 * T, SG], [1, T]])
                        (nc.sync if d == 1 else nc.scalar).dma_start(
                            out=dstd, in_=srcd)
                with nc.allow_low_precision("y in bf16"):
                    nc.vector.tensor_mul(da, da, cb)
                    # reduce 4 state planes -> 2 -> 1
                    nc.vector.tensor_add(
                        rep(da, [[dpit, 128], [BLK, 2], [1, BLK]]),
                        rep(da, [[dpit, 128], [BLK, 2], [1, BLK]]),
                        rep(AP(da.tensor, da.offset + 2 * BLK, da.ap),
                            [[dpit, 128], [BLK, 2], [1, BLK]]))
                y3 = rep(yacc, [[yacc.ap[0][0], 128], [T, 2], [1, T]])
                b0 = rep(da, [[dpit, 128], [T + 1, 2], [1, T]])
                b1 = rep(AP(da.tensor, da.offset + BLK, da.ap),
                         [[dpit, 128], [T + 1, 2], [1, T]])
                if g == 0:
                    nc.vector.tensor_add(y3, b0, b1)
                else:
                    nc.vector.tensor_add(b0, b0, b1)
                    nc.vector.tensor_add(y3, y3, b0)

            # ---- y = (xs*D + yacc) * zsilu ; out_proj + residual ----------
            Drep = rep(blobf[:, F(f"D{l}"):], [[ppitch, 128], [1, 2], [0, T]])
            xs3 = rep(xs, [[xs.ap[0][0], 128], [T, 2], [1, T]])
            ya3 = rep(yacc, [[yacc.ap[0][0], 128], [T, 2], [1, T]])
            yf = act.tile([128, 2 * T], dt16, tag="xsig")
            yf3 = rep(yf, [[yf.ap[0][0], 128], [T, 2], [1, T]])
            nc.gpsimd.tensor_tensor(yf3, xs3, Drep, op=Alu.mult)
            with nc.allow_low_precision("y in bf16"):
                nc.vector.tensor_add(yf, yf, yacc)
                nc.gpsimd.tensor_mul(yf, yf, zsilu)
            res_new = sb.tile([128, T], dt16, tag="res", name=f"res{l}")
            for d in range(2):
                ow = blobh[:, OFF_H[f"outw{d}{l}"]:OFF_H[f"outw{d}{l}"] + D_MODEL]
                for c in range(NCH):
                    sj = slice(c * MM, (c + 1) * MM)
                    po = pp.tile([D_MODEL, MM], dt32, tag="pp")
                    nc.tensor.matmul(po, ow, yf[:, d * T + c * MM:
                                                d * T + (c + 1) * MM],
                                     start=True, stop=True)
                    nc.vector.tensor_add(
                        res_new[d * D_MODEL:(d + 1) * D_MODEL, sj], po,
                        res[d * D_MODEL:(d + 1) * D_MODEL, sj])
            res = res_new

        # ---- head: final LN, softmax pool, linear -------------------------
        if do_head:
            hf = layer_norm(res, F("nfw"), F("nfwm"), F("nfb"), dt16, "xpad")
            logits = rows.tile([2, T], dt16, tag="logits")
            plhs = blobh[:, OFF_H["pool"]:OFF_H["pool"] + 2]
            for c in range(NCH):
                sj = slice(c * MM, (c + 1) * MM)
                pl = pbc.tile([2, MM], dt32, tag="pstat")
                nc.tensor.matmul(pl, plhs, hf[:, sj], start=True, stop=True)
                nc.scalar.activation(logits[:, sj], pl, Act.Copy)
            sm = rows.tile([2, 4], dt32, tag="sm")
            nc.vector.reduce_max(sm[:, 0:1], logits, axis=mybir.AxisListType.X)
            nc.vector.tensor_scalar_mul(sm[:, 1:2], sm[:, 0:1], -1.0)
            nc.scalar.activation(logits, logits, Act.Exp, bias=sm[:, 1:2])
            nc.vector.reduce_sum(sm[:, 2:3], logits, axis=mybir.AxisListType.X)
            nc.vector.reciprocal(sm[:, 3:4], sm[:, 2:3])
            with nc.allow_low_precision("softmax weights bf16"):
                nc.vector.tensor_scalar_mul(logits, logits, sm[:, 3:4])
            bc64 = rep(blobh[0:2, OFF_H["bcast64"]:], [[hpitch, 2], [1, 128]])
            for c in range(NCH):
                sj = slice(c * MM, (c + 1) * MM)
                ab = pbc.tile([128, MM], dt32, tag="pb")
                nc.tensor.matmul(ab, bc64, logits[:, sj], start=True, stop=True)
                with nc.allow_low_precision("pool in bf16"):
                    nc.vector.tensor_mul(hf[:, sj], hf[:, sj], ab)
            pooled = rows.tile([128, 1], dt16, tag="pooled")
            with nc.allow_low_precision("pool in bf16"):
                nc.vector.reduce_sum(pooled, hf, axis=mybir.AxisListType.X)
            pout = pp.tile([D_MODEL, 1], dt32, tag="pp")
            nc.tensor.matmul(pout,
                             blobh[:, OFF_H["ll_wT"]:OFF_H["ll_wT"] + 64],
                             pooled, start=True, stop=True)
            out_sb = rows.tile([D_MODEL, 1], dt32, tag="outsb")
            nc.scalar.activation(out_sb, pout, Act.Identity,
                                 bias=blobf[0:64, F("ll_b"):F("ll_b") + 1])
            nc.sync.dma_start(out=out_d, in_=out_sb)
        else:
            out_sb = rows.tile([D_MODEL, 1], dt32, tag="outsb")
            nc.vector.tensor_copy(out_sb, res[0:D_MODEL, 0:1])
            nc.sync.dma_start(out=out_d, in_=out_sb)

    if legalize:
        _legalize_sync_waits(nc, mybir)
    return nc


def prep_inputs(inputs):
    import ml_dtypes
    f = np.float32
    x = np.asarray(inputs["x"], f).reshape(B, D_MODEL, T)
    xb = x[:, :, ::-1]

    blobf = np.zeros((128, NBF), f)
    O = OFF_F
    blobf[0:64, O["ll_b"]] = np.asarray(inputs["ll_b"], f)
    blobf[0:64, O["nfw"]] = np.asarray(inputs["nf_w"], f)
    blobf[64:128, O["nfw"]] = np.asarray(inputs["nf_w"], f)
    blobf[:, O["nfwm"]] = -blobf[:, O["nfw"]]
    blobf[0:64, O["nfb"]] = np.asarray(inputs["nf_b"], f)
    blobf[64:128, O["nfb"]] = np.asarray(inputs["nf_b"], f)
    nw, nb = np.asarray(inputs["nw"], f), np.asarray(inputs["nb"], f)
    conv_w = np.asarray(inputs["conv_w"], f)
    conv_b = np.asarray(inputs["conv_b"], f)
    dt_b = np.asarray(inputs["dt_b"], f)
    A = -np.exp(np.asarray(inputs["A_log"], f))       # [2,4,128,16]
    Dp = np.asarray(inputs["D"], f)
    for l in range(N_LAYER):
        blobf[0:64, O[f"nw{l}"]] = nw[0, l]
        blobf[64:128, O[f"nw{l}"]] = nw[1, l]
        blobf[:, O[f"nwm{l}"]] = -blobf[:, O[f"nw{l}"]]
        blobf[0:64, O[f"nb{l}"]] = nb[0, l]
        blobf[64:128, O[f"nb{l}"]] = nb[1, l]
        for d in range(2):
            blobf[:, O[f"convw{l}"] + 4 * d:O[f"convw{l}"] + 4 * d + 4] = \
                conv_w[d, l]
            blobf[:, O[f"convb{l}"] + d] = conv_b[d, l]
            blobf[:, O[f"dtb{l}"] + d] = dt_b[d, l]
            blobf[:, O[f"A{l}"] + 16 * d:O[f"A{l}"] + 16 * d + 16] = A[d, l]
            blobf[:, O[f"D{l}"] + d] = Dp[d, l]

    blobh = np.zeros((128, NBH), f)
    H = OFF_H
    blobh[0:64, H["lnsel"]] = 1.0 / D_MODEL
    blobh[64:128, H["lnsel"] + 1] = 1.0 / D_MODEL
    blobh[0, H["bcast64"]:H["bcast64"] + 64] = 1.0
    blobh[1, H["bcast64"] + 64:H["bcast64"] + 128] = 1.0
    blobh[0:64, H["pool"]] = np.asarray(inputs["fp_w"], f)[0]
    blobh[64:128, H["pool"] + 1] = np.asarray(inputs["bp_w"], f)[0]
    blobh[:, H["ll_wT"]:H["ll_wT"] + 64] = np.asarray(inputs["ll_w"], f).T
    in_w = np.asarray(inputs["in_w"], f)              # [2,4,256,64]
    xproj_w = np.asarray(inputs["xproj_w"], f)        # [2,4,36,128]
    dt_w = np.asarray(inputs["dt_w"], f)              # [2,4,128,4]
    out_w = np.asarray(inputs["out_w"], f)            # [2,4,64,128]
    for l in range(N_LAYER):
        for d in range(2):
            blobh[:, H[f"A16{l}"] + 16 * d:H[f"A16{l}"] + 16 * d + 16] = A[d, l]
    for l in range(N_LAYER):
        blobh[0:64, H[f"inw{l}"]:H[f"inw{l}"] + 256] = in_w[0, l].T
        blobh[64:128, H[f"inw{l}"]:H[f"inw{l}"] + 256] = in_w[1, l].T
        for d in range(2):
            xp = xproj_w[d, l].T                      # [128, 36]
            blobh[:, H[f"xproj{d}{l}"]:H[f"xproj{d}{l}"] + 16] = \
                xp[:, DT_RANK:DT_RANK + D_STATE]
            blobh[:, H[f"xproj{d}{l}"] + 32:H[f"xproj{d}{l}"] + 48] = \
                xp[:, DT_RANK + D_STATE:]
            blobh[:, H[f"xproj{d}{l}"] + 64:H[f"xproj{d}{l}"] + 68] = \
                xp[:, 0:DT_RANK]
            blobh[64:68, H[f"dtw{d}{l}"]:H[f"dtw{d}{l}"] + D_INNER] = \
                dt_w[d, l].T
            blobh[:, H[f"outw{d}{l}"]:H[f"outw{d}{l}"] + D_MODEL] = \
                out_w[d, l].T
    blobh16 = blobh.astype(ml_dtypes.bfloat16)

    in_maps = []
    for b in range(B):
        bf = blobf.copy()
        bf[0:64, 0:T] = x[b]
        bf[64:128, 0:T] = xb[b]
        in_maps.append({"blobf": bf, "blobh": blobh16})
    return in_maps


def kernel(**inputs):
    from concourse.bass_utils import run_bass_kernel_spmd
    in_maps = prep_inputs(inputs)
    nc = build_nc()
    res = run_bass_kernel_spmd(nc, in_maps, core_ids=list(range(NCORES)))
    out = np.stack([res.results[b]["out"][:, 0] for b in range(B)])
    return out.astype(np.float32)
